# revision 44
# baseline (speedup 1.0000x reference)
"""Trainium2 Bass kernel for nn_ODEG_8942121911067 (gnn_message_passing).

Math (derived from the reference ODE block; the Euler loop collapses to
its last step since f is recomputed from x_aug every iteration):

    out = relu(0.5*x_aug + 0.125*sigmoid(alpha)_i * (adj @ x_aug)
               + 0.25*S*R + 0.25*(x_aug @_t W2mix))

with x_aug = concat([x, zeros10], -1), S[b,n,t] = sum_f x_aug[b,n,t,f],
R[m] = sum_n ((w*clip(d,0,1)) @ w.T)[m,n], W2mix = (w2*clip(d2,0,1)) @ w2.T.

The 10 zero-padding output columns are relu(0.25*S*R[64:74]) - a rank-1
outer product with no adjacency/temporal coupling - and are filled on
the host; the device computes the 64 real columns.

Device strategy (data-parallel over batch, 4 batches/core on 8 cores):
  - The node-mixing term runs as K=512 PSUM-accumulated matmuls on the
    PE with stationary A = diag(sigmoid(alpha)/8) @ adj (host-built).
    A and x travel as fp8e4 with DoubleRow perf mode (K=256/matmul).
    A is pre-scaled by a power of two into fp8 range (raw entries
    ~1e-4 would flush as subnormals); the descale rides the ACT
    eviction scale as a per-partition input. The adjacency term is ~1%
    of the output magnitude, so fp8 rounding there is negligible.
  - All precision-critical linear terms (0.5*x, the temporal T=24 mix,
    and the rank-1 S*R body term - all layout-hostile to the PE but <5%
    of FLOPs) fold host-side into one bf16 side tensor q, pre-scaled by
    the same power of two. A bf16 identity matmul accumulates q into
    the same PSUM bank as the adjacency chain, so the ACT engine evicts
    each [128, 24*64] PSUM block once with relu(psum * descale) -> bf16.
  - The kernel is at the HBM roofline (~16 MB/core): inputs prefetch
    up front on the gpsimd+scalar DMA rings (batch-interleaved), the
    sync ring carries only outputs so evictions never queue behind
    prefetches, and the PE/ACT both fit under the DMA time.
"""

import numpy as np

B, N, T, F = 32, 512, 24, 64
NUM_ZEROS = 10
FA = F + NUM_ZEROS  # 74
N_CORES = 8
BPC = B // N_CORES  # batches per core = 4
NT = N // 128  # node chunks = 4
NCH = (T * F) // 512  # moving-dim chunks of 512 = 3
TPC = 512 // F  # t-values per 512-chunk = 8

_CACHE = {}


def _build():
    import concourse.mybir as mybir
    import concourse.tile as tile
    from concourse import bacc

    bf16 = mybir.dt.bfloat16
    fp8 = mybir.dt.float8e4
    f32 = mybir.dt.float32

    nc = bacc.Bacc("TRN2", target_bir_lowering=False, debug=False,
                   num_devices=N_CORES)
    x_d = nc.dram_tensor("xin", [BPC, N, T * F], fp8, kind="ExternalInput").ap()
    q_d = nc.dram_tensor("q", [BPC, N, T * F], bf16, kind="ExternalInput").ap()
    at_d = nc.dram_tensor("at", [N, N], fp8, kind="ExternalInput").ap()
    id_d = nc.dram_tensor("idm", [128, 128], bf16, kind="ExternalInput").ap()
    sc_d = nc.dram_tensor("sc", [128, 1], f32, kind="ExternalInput").ap()
    out_d = nc.dram_tensor("out", [BPC, N, T * F], bf16, kind="ExternalOutput").ap()

    with tile.TileContext(nc) as tc:
        with (
            tc.tile_pool(name="const", bufs=1) as cpool,
            tc.tile_pool(name="xp", bufs=BPC) as xpool,
            tc.tile_pool(name="qp", bufs=2 * BPC) as qpool,
            tc.tile_pool(name="op", bufs=8) as opool,
            tc.tile_pool(name="ps", bufs=2, space="PSUM") as pspool,
        ):
            H = NT // 2
            # Prefetch order puts block 0's critical inputs (the ic=0
            # quarter of at, x(b0), q(b0,h0)) at the head of both input
            # rings so the pipeline fills fast; the sync ring is
            # reserved for outputs so evictions never queue behind
            # these. x is one merged transfer per batch; at loads in
            # per-ic column quarters.
            atile = cpool.tile([128, NT, N], fp8, tag="at")
            atv = at_d[:].rearrange("(c p) n -> p c n", p=128)
            xts = {}
            qts = {}

            def load_x(b, eng):
                xh = xpool.tile([128, NT, T * F], fp8, tag="xt")
                eng.dma_start(
                    xh[:], x_d[b].rearrange("(c p) tf -> p c tf", p=128))
                xts[b] = xh

            def load_q(b, h, eng):
                qh = qpool.tile([128, H, T * F], bf16, tag="qt")
                eng.dma_start(
                    qh[:],
                    q_d[b].rearrange("(h c p) tf -> h p c tf", h=2, p=128)[h])
                qts[b, h] = qh

            # b0's x rides both rings as h-halves so the first chain's
            # inputs land in parallel with at; x0h1 takes the sync ring,
            # which is otherwise idle until outputs start.
            nc.scalar.dma_start(atile[:], atv)
            x0 = xpool.tile([128, NT, T * F], fp8, tag="xt")
            xv0 = x_d[0].rearrange("(h c p) tf -> h p c tf", h=2, p=128)
            nc.gpsimd.dma_start(x0[:, 0:2], xv0[0])
            nc.sync.dma_start(x0[:, 2:4], xv0[1])
            xts[0] = x0
            load_q(0, 0, nc.sync)
            idt = cpool.tile([128, 128], bf16, tag="idm")
            nc.gpsimd.dma_start(idt[:], id_d[:])
            sc = cpool.tile([128, 1], f32, tag="sc")
            nc.gpsimd.dma_start(sc[:], sc_d[:])
            load_q(0, 1, nc.scalar)
            for b in range(1, BPC):
                load_x(b, nc.gpsimd)
                load_q(b, 0, nc.gpsimd)
                load_q(b, 1, nc.scalar)

            for b in range(BPC):
                for ic in range(NT):
                    qt = qts[b, ic // H][:, ic % H]
                    # Alternate eviction engines so ACT and DVE each
                    # carry half the blocks and neither paces the loop.
                    dve_evict = (b * NT + ic) % 2 == 1
                    ot = opool.tile([128, T * F], bf16, tag="ot")
                    ps = pspool.tile([128, T * F], f32, tag="ps")
                    for nch in range(NCH):
                        c0 = nch * 512
                        for h in range(2):
                            nc.tensor.matmul(
                                ps[:, c0:c0 + 512],
                                atile[:, 2 * h:2 * h + 2,
                                      ic * 128:(ic + 1) * 128],
                                xts[b][:, 2 * h:2 * h + 2, c0:c0 + 512],
                                start=(h == 0),
                                stop=(h == 1 and dve_evict),
                                perf_mode=mybir.MatmulPerfMode.DoubleRow,
                                skip_group_check=True,
                            )
                        if not dve_evict:
                            # identity matmul carries sq so q stays raw
                            nc.tensor.matmul(
                                ps[:, c0:c0 + 512],
                                idt[:],
                                qt[:, c0:c0 + 512],
                                start=False,
                                stop=True,
                                skip_group_check=True,
                            )
                    if dve_evict:
                        nc.vector.scalar_tensor_tensor(
                            ot[:], ps[:], sc[:, 0:1], qt[:],
                            mybir.AluOpType.mult,
                            mybir.AluOpType.add,
                        )
                        nc.vector.tensor_scalar_max(ot[:], ot[:], 0.0)
                    else:
                        nc.scalar.activation(
                            ot[:], ps[:],
                            mybir.ActivationFunctionType.Relu,
                            scale=sc[:, 0:1])
                    od = out_d[b, ic * 128:(ic + 1) * 128]
                    if b == BPC - 1 and ic == NT - 1:
                        # split the final eviction across two idle rings
                        nc.gpsimd.dma_start(od[0:64], ot[0:64])
                        nc.scalar.dma_start(od[64:128], ot[64:128])
                    else:
                        oeng = (nc.gpsimd if (b >= 2 and ic % 2 == 1)
                                else nc.sync)
                        oeng.dma_start(od, ot[:])

    nc.compile()
    return nc


def prepare(x, adj, alpha, w, d, w2, d2):
    """Host prep: fold parameters, build q. Returns (nc, in_maps)."""
    import ml_dtypes

    fp8 = ml_dtypes.float8_e4m3

    x = np.ascontiguousarray(np.asarray(x), np.float32)
    adj = np.asarray(adj)
    alpha = np.asarray(alpha)
    w = np.asarray(w)
    d = np.asarray(d)
    w2 = np.asarray(w2)
    d2 = np.asarray(d2)
    a = 1.0 / (1.0 + np.exp(-alpha.astype(np.float32)))
    A = 0.125 * a[:, None] * adj.astype(np.float32)

    # fp8e4 (e4m3, max 240): scale A and, if needed, x into range by
    # powers of two; the product of the inverses descales the PSUM.
    amax = max(float(np.abs(A).max()), 1e-30)
    sa = 2.0 ** np.floor(np.log2(120.0 / amax))
    xmax = max(float(np.abs(x).max()), 1e-30)
    sx = 2.0 ** min(np.floor(np.log2(120.0 / xmax)), 0.0)
    at = np.ascontiguousarray(A.T * sa, dtype=fp8)
    xb = ((x * sx) if sx != 1.0 else x).astype(fp8)
    sq = np.float32(sa * sx)
    sc = np.full((128, 1), 1.0 / sq, np.float32)
    idm = (np.eye(128, dtype=np.float32) * sq).astype(ml_dtypes.bfloat16)

    dc = np.clip(d.astype(np.float32), 0.0, 1.0)
    W = (w.astype(np.float32) * dc) @ w.astype(np.float32).T
    R = W.sum(axis=1)  # [FA]
    d2c = np.clip(d2.astype(np.float32), 0.0, 1.0)
    W2 = (w2.astype(np.float32) * d2c) @ w2.astype(np.float32).T  # [T,T]

    S = x.sum(axis=3)  # [B,N,T]

    # q = 0.5*x + 0.25*(x @_t W2) + 0.25*S*R[:64], kept raw: the
    # identity matmul's sq*I stationary lifts it onto the scaled fp8
    # adjacency sum for ACT-evicted blocks, and the DVE path adds it
    # after its own descale.
    xt = np.matmul(x.transpose(0, 1, 3, 2), 0.25 * W2)  # [B,N,F,T]
    q = np.ascontiguousarray(xt.transpose(0, 1, 3, 2))
    q += 0.5 * x
    q += 0.25 * S[..., None] * R[:F]
    q = q.astype(ml_dtypes.bfloat16)

    # Host-side pad columns: relu(0.25 * S * R[64:74])
    pads = np.maximum(0.25 * S[..., None] * R[F:], 0.0).astype(np.float32)
    _CACHE["pads"] = pads

    if "nc" not in _CACHE:
        _CACHE["nc"] = _build()
    nc = _CACHE["nc"]
    xb = xb.reshape(B, N, T * F)
    q = q.reshape(B, N, T * F)
    in_maps = [
        {"xin": xb[c * BPC:(c + 1) * BPC], "q": q[c * BPC:(c + 1) * BPC],
         "at": at, "idm": idm, "sc": sc}
        for c in range(N_CORES)
    ]
    return nc, in_maps


def assemble(results):
    """Concatenate per-core outputs, upcast, and add host pad columns."""
    dev = np.concatenate([results[c]["out"] for c in range(N_CORES)], axis=0)
    out = np.empty((B, N, T, FA), np.float32)
    out[..., :F] = dev.reshape(B, N, T, F).astype(np.float32)
    out[..., F:] = _CACHE["pads"]
    return out


def kernel(x, adj, alpha, w, d, w2, d2):
    from concourse.bass_utils import run_bass_kernel_spmd

    nc, in_maps = prepare(x, adj, alpha, w, d, w2, d2)
    res = run_bass_kernel_spmd(nc, in_maps, list(range(N_CORES)))
    return assemble(res.results)
